# revision 73
# baseline (speedup 1.0000x reference)
"""AttentionBlock (GroupNorm + single-head NxN attention + residual) on 8 TRN2 cores.

Data-parallel: batch dim (B=8) sharded 1 image per NeuronCore. Per core:

  x (C=256, N=4096) f32 -> GroupNorm stats (vector row-sums + scalar
  square-accum, pipelined behind the x DMA); the GN affine is FOLDED INTO
  THE QKV WEIGHTS: w' = fp8(w_bf16 * a[cin]) with a = rstd*gn_scale, and
  the mean/bias offset is restored through tiny on-device bias matmuls
  (beta = w' @ (b_full/a) + b_host; the v-path bias passes through softmax
  and lands in the output bias). x is cast to raw fp8 once.

  All heavy matmuls are fp8e4m3 DoubleRow (contraction 256 = full C, or an
  m-pair of two 128-key tiles, in ONE matmul at 2 MACs/cell/cycle):
    k = Wk' x8, q = Wq' x8 (q produced one nch ahead, inside the body)
    s = k^T q  -> e = exp(s/16 - 4) fp8 (one paired exp per two key tiles;
    the scalar engine runs Exp back-to-back and is the body bottleneck)
    attn_u = v @ e, den = ones @ e, proj_u = Wo @ attn_u
    out = proj_u * (1/den) + b_out_eff + x

  The exponent shift -4 keeps exp under the fp8 max (240) and cancels in
  the normalization. Softmax rows sum to 1, so v/out biases fold exactly.

Schedule: a flat (nch, key-pair) software pipeline; AV/den matmuls and the
per-nch tail (atts cast, reciprocal, projection, residual, store) trail the
scores/exp stream via a deferral queue so the in-order PE never waits on
the PSUM handoffs. PSUM: scores-pair 2x2 banks, attn 2, den 1, v/proj/q 1.
"""

import sys

if "/opt/trn_rl_repo" not in sys.path:
    sys.path.insert(0, "/opt/trn_rl_repo")

import numpy as np

import concourse.bass as bass
import concourse.bacc as bacc
import concourse.tile as tile
import concourse.mybir as mybir
from concourse import bass_utils

# Problem dims (hardcoded per spec)
B, C, HH, WW = 8, 256, 64, 64
N = HH * WW            # 4096
G = 8                  # groupnorm groups
GSZ = C // G           # 32 channels/group
EPS = 1e-5
P = 128                # SBUF partitions
CT = C // P            # 2 channel tiles
NCH = 512              # query-chunk width (free dim per matmul)
NNCH = N // NCH        # 8
MT = N // P            # 32 key tiles
MP = MT // 2           # 16 key-tile pairs
SCALE = 1.0 / np.sqrt(C)
ESHIFT = -4.0          # exponent shift; cancels in normalization
INV_CNT = 1.0 / (GSZ * N)

F32 = mybir.dt.float32
F32R = mybir.dt.float32r
FP8 = mybir.dt.float8e4
DR = mybir.MatmulPerfMode.DoubleRow


def _emit(tc, d, out_d):
    from contextlib import ExitStack

    nc = tc.nc
    AF = mybir.ActivationFunctionType
    OP = mybir.AluOpType
    AX = mybir.AxisListType.X
    ts, ds = bass.ts, bass.ds

    with ExitStack() as ctx:
        const = ctx.enter_context(tc.tile_pool(name="const", bufs=1))
        big = ctx.enter_context(tc.tile_pool(name="big", bufs=1))
        work = ctx.enter_context(tc.tile_pool(name="work", bufs=3))
        small = ctx.enter_context(tc.tile_pool(name="small", bufs=4))
        outp = ctx.enter_context(tc.tile_pool(name="outp", bufs=3))
        # PSUM: 8 banks total. s-pair 2 bufs x 2 banks, attn 2 banks,
        # den 1 bank, v/proj shared 1 bank.
        psS = ctx.enter_context(tc.tile_pool(name="psS", bufs=2, space="PSUM"))
        psA = ctx.enter_context(tc.tile_pool(name="psA", bufs=1, space="PSUM"))
        psD = ctx.enter_context(tc.tile_pool(name="psD", bufs=1, space="PSUM"))
        psP = ctx.enter_context(tc.tile_pool(name="psP", bufs=1, space="PSUM"))

        # ---------------- DMAs: packed consts first, then x on 4 queues -----
        # consts_a columns: b_q(2) b_k(2) b_o(2) gn_w(2) gn_b(2) fmask(2x8)
        ca = const.tile([P, 26], F32, name="ca")
        nc.scalar.dma_start(out=ca, in_=d["consts_a"])
        bm_sb = const.tile([G, CT, P], F32, name="bm_sb")
        nc.scalar.dma_start(out=bm_sb[:, :, :], in_=d["bmask"])
        BQ, BK, BO = 0, 2, 4         # ca column offsets

        # x on the sync/gpsimd queues only (the scalar queue carries the small
        # consts + fp8 weights and must stay clear for the GN squares)
        NC4 = 4                      # head chunks per channel-tile
        CW = N // NC4                # 1024 columns per chunk
        BF16 = mybir.dt.bfloat16
        xb_sb = big.tile([P, CT, N], BF16, name="xb_sb")
        x_sb = big.tile([P, CT, N], F32, name="x_sb")
        xq = [nc.sync, nc.gpsimd, nc.scalar]
        for c in range(NC4):
            for t in range(CT):
                csl = ds(c * CW, CW)
                xq[(c * CT + t) % 3].dma_start(out=xb_sb[:, t, csl],
                                               in_=d["x_bf"][ts(t, P), csl])

        # qkv weights arrive bf16 pair-packed ([cin_half, 2, cout]); the GN
        # per-channel scale is folded into them on-device -> fp8. wo is fp8.
        wqb_sb = const.tile([P, CT, C], BF16, name="wqb_sb")
        wkb_sb = const.tile([P, CT, C], BF16, name="wkb_sb")
        wvb_sb = const.tile([P, CT, C], BF16, name="wvb_sb")
        wo_sb = const.tile([P, CT, C], FP8, name="wo_sb")
        for sb, dr in ((wkb_sb, d["wkb"]), (wqb_sb, d["wqb"]),
                       (wvb_sb, d["wvb"]), (wo_sb, d["wo8"])):
            nc.scalar.dma_start(out=sb, in_=dr)
        wq_sb = const.tile([P, CT, C], FP8, name="wq_sb")
        wk_sb = const.tile([P, CT, C], FP8, name="wk_sb")
        wv_sb = const.tile([P, CT, C], FP8, name="wv_sb")

        # fp8 ones pair for the den matmul (pair-dim stride kept at 16B)
        ones_sb = const.tile([P, CT, 16], FP8, name="ones_sb")
        nc.gpsimd.memset(ones_sb, 1.0)
        zero_sb = const.tile([P, 1], F32, name="zero_sb")
        nc.gpsimd.memset(zero_sb, 0.0)
        zerob_sb = const.tile([P, 1], BF16, name="zerob_sb")
        nc.gpsimd.memset(zerob_sb, 0.0)
        esh_sb = const.tile([P, 1], F32, name="esh_sb")
        nc.gpsimd.memset(esh_sb, ESHIFT)
        eps_sb = const.tile([G, 1], F32, name="eps_sb")
        nc.gpsimd.memset(eps_sb, EPS)

        # ---------------- GroupNorm stats (pipelined behind x DMA) ----------
        # Both row-sum and x^2 row-sum run on vector (tensor_tensor_reduce),
        # keeping the scalar engine's activation-table sequence to
        # Sqrt/Identity -> Exp with both loads preloaded off the critical
        # path (dummy ops below).
        x8_sb = big.tile([P, CT, N], FP8, name="x8_sb")  # raw x cast to fp8
        sq_scr = small.tile([P, 2, CW], F32, name="sq_scr")
        stat = small.tile([P, CT, NC4, 2], F32, name="stat")
        gps = psS.tile([G, 2], F32, tag="s", name="gps")
        for c in range(NC4):
            for t in range(CT):
                csl = ds(c * CW, CW)
                nc.vector.reduce_sum(out=stat[:, t, c, 0:1], in_=xb_sb[:, t, csl],
                                     axis=AX)
                nc.scalar.activation(out=sq_scr[:, (c * CT + t) % 2, :],
                                     in_=xb_sb[:, t, csl],
                                     func=AF.Square, bias=zero_sb,
                                     accum_out=stat[:, t, c, 1:2])
                nc.vector.tensor_copy(out=x8_sb[:, t, csl], in_=xb_sb[:, t, csl])
                # PE warm-up during the head: short bf16 matmul on the chunk
                if t == 0:
                    warm = psS.tile([1, P], F32, tag="s", name="warm")
                    nc.tensor.matmul(warm, lhsT=zerob_sb,
                                     rhs=xb_sb[:, t, ds(c * CW, P)],
                                     start=True, stop=True)
            for t in range(CT):
                nc.tensor.matmul(gps, lhsT=ca[:, ds(10 + G * t, G)],
                                 rhs=stat[:, t, c, :],
                                 start=(c == 0 and t == 0),
                                 stop=(c == NC4 - 1 and t == CT - 1))
            if c == 1:
                # preload the Sqrt/Identity table in a DMA-wait gap; the
                # Square reload after it also hides in the stats pipeline
                nc.scalar.activation(out=eps_sb, in_=eps_sb, func=AF.Sqrt,
                                     bias=eps_sb)
                nc.gpsimd.memset(eps_sb, EPS)
        for w in range(8):
            warm = psS.tile([1, NCH], F32, tag="s", name="warmb")
            nc.tensor.matmul(warm, lhsT=zerob_sb,
                             rhs=xb_sb[:, w % 2, ds(((w * 3) % 7) * NCH, NCH)],
                             start=True, stop=True)
        grp = small.tile([G, 2], F32, name="grp")    # [mean, rstd]
        gtmp = small.tile([G, 3], F32, name="gtmp")
        nc.vector.tensor_scalar_mul(out=grp[:, 0:1], in0=gps[:, 0:1], scalar1=INV_CNT)
        nc.vector.tensor_scalar_mul(out=gtmp[:, 0:1], in0=gps[:, 1:2], scalar1=INV_CNT)
        nc.vector.tensor_mul(out=gtmp[:, 1:2], in0=grp[:, 0:1], in1=grp[:, 0:1])
        nc.vector.tensor_sub(out=gtmp[:, 2:3], in0=gtmp[:, 0:1], in1=gtmp[:, 1:2])
        nc.scalar.activation(out=gtmp[:, 2:3], in_=gtmp[:, 2:3], func=AF.Sqrt,
                             bias=eps_sb)
        nc.vector.reciprocal(out=grp[:, 1:2], in_=gtmp[:, 2:3])

        ab = small.tile([P, CT, 2], F32, name="ab")  # per-channel scale a, bias b
        for t in range(CT):
            cps = psS.tile([P, 2], F32, tag="s", name="cps")
            nc.tensor.matmul(cps, lhsT=bm_sb[:, t, :], rhs=grp, start=True, stop=True)
            nc.vector.tensor_mul(out=ab[:, t, 0:1], in0=cps[:, 1:2], in1=ca[:, 6 + t:7 + t])
            nc.vector.tensor_mul(out=ab[:, t, 1:2], in0=cps[:, 0:1], in1=ab[:, t, 0:1])
            nc.vector.tensor_sub(out=ab[:, t, 1:2], in0=ca[:, 8 + t:9 + t], in1=ab[:, t, 1:2])


        # ------- fold GN affine into the weights + device-side bias terms ---
        # w' = fp8(w_bf16 * a[cin]);  beta = w' @ (b_full/a) + b_host.
        # The v-path bias flows through softmax (rows sum to 1) into the
        # output: delta = Wo @ (Wv' bvec); combined with the host b_o fold.
        q_sb = big.tile([P, CT, N], FP8, name="q_sb")   # (c_half, n)
        k_sb = big.tile([P, CT, N], FP8, name="k_sb")
        vT_sb = big.tile([P, MT, C], FP8, name="vT_sb")  # (n, c), n on partitions

        rab = small.tile([P, CT, 2], F32, name="rab")   # [1/a, b_full/a]
        nc.vector.reciprocal(out=rab[:, :, 0:1], in_=ab[:, :, 0:1])
        nc.vector.tensor_mul(out=rab[:, :, 1:2], in0=ab[:, :, 1:2], in1=rab[:, :, 0:1])
        bvec8 = small.tile([P, CT, 16], FP8, name="bvec8")
        nc.vector.tensor_copy(out=bvec8[:, :, 0:1], in_=rab[:, :, 1:2])

        for j, (w8, wb) in enumerate(((wk_sb, wkb_sb), (wq_sb, wqb_sb),
                                      (wv_sb, wvb_sb))):
            for t in range(CT):
                if (j + t) % 2 == 0:
                    nc.vector.tensor_scalar_mul(out=w8[:, t, :], in0=wb[:, t, :],
                                                scalar1=ab[:, t, 0:1])
                else:
                    nc.scalar.activation(out=w8[:, t, :], in_=wb[:, t, :],
                                         func=AF.Identity, bias=zero_sb,
                                         scale=ab[:, t, 0:1])

        # tiny DR matmuls for the bias terms, all into one psP bank
        bps = psP.tile([P, NCH], F32, tag="proj", name="bps")
        for i, w8 in enumerate((wq_sb, wk_sb)):
            for tq in range(CT):
                nc.tensor.matmul(bps[:, 2 * i + tq:2 * i + tq + 1],
                                 lhsT=w8[:, :, ts(tq, P)], rhs=bvec8[:, :, 0:1],
                                 start=True, stop=True, perf_mode=DR)
        beta = small.tile([P, 6], F32, name="beta")     # [q0 q1 k0 k1 o0 o1]
        nc.vector.tensor_add(out=beta[:, 0:4], in0=bps[:, 0:4], in1=ca[:, 0:4])

        def _vbias():
            for tq in range(CT):
                nc.tensor.matmul(bps[:, 4 + tq:5 + tq],
                                 lhsT=wv_sb[:, :, ts(tq, P)], rhs=bvec8[:, :, 0:1],
                                 start=True, stop=True, perf_mode=DR)
            usb8 = small.tile([P, CT, 16], FP8, name="usb8")
            for t in range(CT):
                nc.vector.tensor_copy(out=usb8[:, t, 0:1], in_=bps[:, 4 + t:5 + t])
            for co in range(CT):
                nc.tensor.matmul(bps[:, 6 + co:7 + co],
                                 lhsT=wo_sb[:, :, ts(co, P)], rhs=usb8[:, :, 0:1],
                                 start=True, stop=True, perf_mode=DR)
            nc.vector.tensor_add(out=beta[:, 4:6], in0=bps[:, 6:8], in1=ca[:, 4:6])

        # K for all slices now; Q is produced one nch ahead inside the body
        # (on the shared psP bank) — only q(nch 0) is made in the head.
        for s in range(NNCH):
            nsl = ds(s * NCH, NCH)
            # alternate the production tile between the psS slots and the
            # (still idle) attn bank pair for a deeper head pipeline
            pool, tag = (psS, "s") if s % 2 == 0 else (psA, "attn")
            pp = pool.tile([P, CT, NCH], F32, tag=tag, name="ppqk")
            for tq in range(CT):
                nc.tensor.matmul(pp[:, tq, :],
                                 lhsT=wk_sb[:, :, ts(tq, P)],
                                 rhs=x8_sb[:, :, nsl],
                                 start=True, stop=True, perf_mode=DR)
            for tq in range(CT):
                eng = nc.vector if (s + tq) % 2 == 0 else nc.scalar
                if eng is nc.vector:
                    nc.vector.tensor_scalar_add(
                        out=k_sb[:, tq, nsl], in0=pp[:, tq, :],
                        scalar1=beta[:, 2 + tq:3 + tq])
                else:
                    nc.scalar.activation(
                        out=k_sb[:, tq, nsl], in_=pp[:, tq, :],
                        func=AF.Identity, bias=beta[:, 2 + tq:3 + tq])

        # full-precision x for the residual: streams in under the body on the
        # sync queue. The DMA engines pull queued descriptors immediately and
        # would steal HBM bandwidth from the head, so gate each chunk behind
        # k-production via a WAW dependency on one element per chunk region.
        nc.gpsimd.tensor_copy(out=x_sb[0:1, :, 0:N:CW], in_=k_sb[0:1, :, 0:N:CW])
        for c in range(NC4):
            for t in range(CT):
                csl = ds(c * CW, CW)
                nc.sync.dma_start(out=x_sb[:, t, csl], in_=d["x"][ts(t, P), csl])

        def _qprod(s, tq, eng):
            nsl = ds(s * NCH, NCH)
            pq = psP.tile([P, NCH], F32, tag="proj", name="pq")
            nc.tensor.matmul(pq, lhsT=wq_sb[:, :, ts(tq, P)],
                             rhs=x8_sb[:, :, nsl],
                             start=True, stop=True, perf_mode=DR)
            eng.tensor_scalar_add(out=q_sb[:, tq, nsl], in0=pq,
                                  scalar1=beta[:, tq:tq + 1])

        pp = psS.tile([P, CT, NCH], F32, tag="s", name="ppq0")
        for tq in range(CT):
            nc.tensor.matmul(pp[:, tq, :], lhsT=wq_sb[:, :, ts(tq, P)],
                             rhs=x8_sb[:, :, 0:NCH],
                             start=True, stop=True, perf_mode=DR)
        for tq in range(CT):
            eng = nc.vector if tq == 0 else nc.scalar
            if tq == 0:
                nc.vector.tensor_scalar_add(out=q_sb[:, tq, 0:NCH],
                                            in0=pp[:, tq, :],
                                            scalar1=beta[:, tq:tq + 1])
            else:
                nc.scalar.activation(out=q_sb[:, tq, 0:NCH], in_=pp[:, tq, :],
                                     func=AF.Identity, bias=beta[:, tq:tq + 1])
        _vbias()
        # preload the Exp activation table before the body needs it
        nc.scalar.activation(out=gtmp[:, 0:1], in_=eps_sb, func=AF.Exp,
                             bias=eps_sb)

        # ---------------- attention + output projection (fp8 DoubleRow) -----
        # v-tile pairs are produced inside nch 0's stream (PSUM bank shared
        # with proj; b_v is folded into b_out host-side).
        # Flat (nch, mp) stream; AV/den trail the scores/exp stream by LAG
        # units ACROSS nch boundaries so the per-nch PSUM handoff (attn CAST,
        # den copy) hides under the next nch's score matmuls.
        LAG = 4
        attns, dens = {}, {}

        tstate = {}

        def _tailA(nch):
            # atts copy FIRST (per channel-half, so half 0 can start as soon
            # as its last AV lands): it releases the attn PSUM pair the next
            # nch's AV matmuls are waiting on. For the final nch the den
            # chain is more urgent than the PSUM release.
            atts = outp.tile([P, CT, NCH], FP8, tag="att", name="atts", bufs=3)
            attp = attns.pop(nch)

            def _cast():
                nc.vector.tensor_copy(out=atts, in_=attp)

            def _den():
                den_sb = small.tile([1, NCH], F32, tag="den_sb", name="den_sb", bufs=2)
                nc.vector.tensor_copy(out=den_sb, in_=dens.pop(nch))
                rden = small.tile([1, NCH], F32, tag="rden", name="rden", bufs=2)
                nc.vector.reciprocal_approx_fast(out=rden, in_=den_sb)
                rdenb = outp.tile([P, NCH], F32, tag="rdenb", name="rdenb", bufs=2)
                nc.gpsimd.partition_broadcast(rdenb, rden)
                return rdenb

            if nch == NNCH - 1:
                rdenb = _den()
                _cast()
            else:
                _cast()
                rdenb = _den()
            tstate[nch] = (atts, rdenb)

        def _tailB(nch):
            nsl = ds(nch * NCH, NCH)
            atts, rdenb = tstate.pop(nch)
            last = nch == NNCH - 1
            # proj per output-channel half on a single PSUM bank; copy to SBUF
            # immediately (PSUM release must not be gated on the rden chain).
            # The final nch has no release pressure: read the PSUM directly
            # and split its stores into halves on both DMA queues.
            for co in range(CT):
                pj = psP.tile([P, NCH], F32, tag="proj", name="pj")
                nc.tensor.matmul(pj, lhsT=wo_sb[:, :, ts(co, P)], rhs=atts,
                                 start=True, stop=True, perf_mode=DR)
                if last:
                    src_pj = pj
                else:
                    src_pj = outp.tile([P, NCH], F32, tag="pjs", name="pjs", bufs=3)
                    nc.vector.tensor_copy(out=src_pj, in_=pj)
                f = outp.tile([P, NCH], F32, tag="fout", name="f", bufs=3)
                nc.vector.tensor_tensor(out=f, in0=src_pj, in1=rdenb, op=OP.mult)
                nc.vector.scalar_tensor_tensor(out=f, in0=f, scalar=beta[:, 4 + co:5 + co],
                                               in1=x_sb[:, co, nsl],
                                               op0=OP.add, op1=OP.add)
                if last:
                    hw = NCH // 2
                    nc.sync.dma_start(out=out_d[ts(co, P), ds(nch * NCH, hw)],
                                      in_=f[:, 0:hw])
                    nc.gpsimd.dma_start(out=out_d[ts(co, P), ds(nch * NCH + hw, hw)],
                                        in_=f[:, hw:NCH])
                else:
                    eng = nc.sync if co == 0 else nc.gpsimd
                    eng.dma_start(out=out_d[ts(co, P), nsl], in_=f)

        def _av(nch, mp, e):
            if mp == 0:
                attns[nch] = psA.tile([P, CT, NCH], F32, tag="attn", name="attn")
                dens[nch] = psD.tile([1, NCH], F32, tag="den", name="den")
            for ch in range(CT):
                nc.tensor.matmul(attns[nch][:, ch, :],
                                 lhsT=vT_sb[:, ds(2 * mp, 2), ts(ch, P)],
                                 rhs=e,
                                 start=(mp == 0), stop=(mp == MP - 1),
                                 perf_mode=DR)
            nc.tensor.matmul(dens[nch], lhsT=ones_sb[:, :, 0:1], rhs=e,
                             start=(mp == 0), stop=(mp == MP - 1),
                             perf_mode=DR)

        def _pop(unit):
            if unit[0] == "tailA":
                _tailA(unit[1])
            elif unit[0] == "tailB":
                _tailB(unit[1])
            else:
                _av(*unit[1:])

        pend = []
        for nch in range(NNCH):
            nsl = ds(nch * NCH, NCH)
            for mp in range(MP):
                if nch == 0:
                    vt = psP.tile([P, CT, C], F32, tag="proj", name="vt")
                    for half in range(2):
                        nc.tensor.matmul(vt[:, half, :],
                                         lhsT=x8_sb[:, :, ts(2 * mp + half, P)],
                                         rhs=wv_sb,
                                         start=True, stop=True, perf_mode=DR)
                    nc.vector.tensor_copy(out=vT_sb[:, ds(2 * mp, 2), :], in_=vt)
                if nch + 1 < NNCH and mp in (10, 11):
                    _qprod(nch + 1, mp - 10, nc.vector)
                sp = psS.tile([P, CT, NCH], F32, tag="s", name="sp")
                for half in range(2):
                    nc.tensor.matmul(sp[:, half, :],
                                     lhsT=k_sb[:, :, ts(2 * mp + half, P)],
                                     rhs=q_sb[:, :, nsl],
                                     start=True, stop=True, perf_mode=DR)
                e = work.tile([P, 2, NCH], FP8, tag="e", name="e", bufs=8)
                nc.scalar.activation(out=e, in_=sp, func=AF.Exp,
                                     bias=esh_sb, scale=SCALE)
                pend.append(("av", nch, mp, e))
                lag = 2 if (nch == NNCH - 1 and mp >= 12) else LAG
                while len(pend) > lag:
                    _pop(pend.pop(0))
                if mp == MP - 1:
                    pend.append(("tailA", nch))
                    pend.append(("tailB", nch))
        for unit in pend:
            _pop(unit)


def build_program():
    nc = bacc.Bacc("TRN2", target_bir_lowering=False, debug=False, num_devices=B)
    d = {}

    def din(name, shape, dt_=F32):
        d[name] = nc.dram_tensor(name, list(shape), dt_, kind="ExternalInput").ap()

    din("x", (C, N))
    din("x_bf", (C, N), mybir.dt.bfloat16)
    din("wqb", (P, CT, C), mybir.dt.bfloat16)
    din("wkb", (P, CT, C), mybir.dt.bfloat16)
    din("wvb", (P, CT, C), mybir.dt.bfloat16)
    din("wo8", (P, CT, C), FP8)
    din("consts_a", (P, 26))
    din("bmask", (G, CT * P))
    out_d = nc.dram_tensor("out", [C, N], F32, kind="ExternalOutput").ap()

    with tile.TileContext(nc) as tc:
        _emit(tc, d, out_d)
    nc.compile()
    return nc


_PROG = None


def _get_program():
    global _PROG
    if _PROG is None:
        _PROG = build_program()
    return _PROG


def make_in_maps(inputs):
    x = np.ascontiguousarray(np.asarray(inputs["x"], dtype=np.float32))
    w_qkv = np.asarray(inputs["w_qkv"], dtype=np.float32)
    b_qkv = np.asarray(inputs["b_qkv"], dtype=np.float32)
    w_out = np.asarray(inputs["w_out"], dtype=np.float32)
    b_out = np.asarray(inputs["b_out"], dtype=np.float32)
    gn_scale = np.asarray(inputs["gn_scale"], dtype=np.float32)
    gn_bias = np.asarray(inputs["gn_bias"], dtype=np.float32)

    fmask = np.zeros((CT, P, G), dtype=np.float32)
    for t in range(CT):
        for p in range(P):
            fmask[t, p, (t * P + p) // GSZ] = 1.0
    # bmask[g, t*P+p] = fmask[t, p, g]
    bmask = np.ascontiguousarray(fmask.transpose(2, 0, 1).reshape(G, CT * P))

    consts_a = np.zeros((P, 26), dtype=np.float32)
    bo_eff = b_out + w_out @ b_qkv[2 * C:3 * C]   # b_v folded (softmax sums to 1)
    for t in range(CT):
        rows = slice(t * P, (t + 1) * P)
        consts_a[:, 0 + t] = b_qkv[0:C][rows]
        consts_a[:, 2 + t] = b_qkv[C:2 * C][rows]
        consts_a[:, 4 + t] = bo_eff[rows]
        consts_a[:, 6 + t] = gn_scale[rows]
        consts_a[:, 8 + t] = gn_bias[rows]
        consts_a[:, 10 + G * t:10 + G * (t + 1)] = fmask[t]

    import ml_dtypes
    E4 = ml_dtypes.float8_e4m3
    BF = ml_dtypes.bfloat16

    def pack(w, dt):
        # [cout, cin] -> lhsT/rhs pair layout [cin_half, 2, cout]
        return np.ascontiguousarray(
            w.T.reshape(CT, P, C).transpose(1, 0, 2)).astype(dt)

    common = {
        "wqb": pack(w_qkv[0:C], BF),
        "wkb": pack(w_qkv[C:2 * C], BF),
        "wvb": pack(w_qkv[2 * C:3 * C], BF),
        "wo8": pack(w_out, E4),
        "consts_a": consts_a,
        "bmask": bmask,
    }
    return [dict(common, x=np.ascontiguousarray(x[b].reshape(C, N)),
                 x_bf=np.ascontiguousarray(x[b].reshape(C, N).astype(BF)))
            for b in range(B)]


def run(inputs, trace=False):
    nc = _get_program()
    in_maps = make_in_maps(inputs)
    res = bass_utils.run_bass_kernel_spmd(nc, in_maps, core_ids=list(range(B)),
                                          trace=trace)
    out = np.stack([res.results[b]["out"] for b in range(B)])
    return out.reshape(B, C, HH, WW), res


def kernel(**inputs):
    out, _ = run(inputs, trace=False)
    return out


# revision 74
# speedup vs baseline: 1.0019x; 1.0019x over previous
"""AttentionBlock (GroupNorm + single-head NxN attention + residual) on 8 TRN2 cores.

Data-parallel: batch dim (B=8) sharded 1 image per NeuronCore. Per core:

  x (C=256, N=4096) f32 -> GroupNorm stats (vector row-sums + scalar
  square-accum, pipelined behind the x DMA); the GN affine is FOLDED INTO
  THE QKV WEIGHTS: w' = fp8(w_bf16 * a[cin]) with a = rstd*gn_scale, and
  the mean/bias offset is restored through tiny on-device bias matmuls
  (beta = w' @ (b_full/a) + b_host; the v-path bias passes through softmax
  and lands in the output bias). x is cast to raw fp8 once.

  All heavy matmuls are fp8e4m3 DoubleRow (contraction 256 = full C, or an
  m-pair of two 128-key tiles, in ONE matmul at 2 MACs/cell/cycle):
    k = Wk' x8, q = Wq' x8 (q produced one nch ahead, inside the body)
    s = k^T q  -> e = exp(s/16 - 4) fp8 (one paired exp per two key tiles;
    the scalar engine runs Exp back-to-back and is the body bottleneck)
    attn_u = v @ e, den = ones @ e, proj_u = Wo @ attn_u
    out = proj_u * (1/den) + b_out_eff + x

  The exponent shift -4 keeps exp under the fp8 max (240) and cancels in
  the normalization. Softmax rows sum to 1, so v/out biases fold exactly.

Schedule: a flat (nch, key-pair) software pipeline; AV/den matmuls and the
per-nch tail (atts cast, reciprocal, projection, residual, store) trail the
scores/exp stream via a deferral queue so the in-order PE never waits on
the PSUM handoffs. PSUM: scores-pair 2x2 banks, attn 2, den 1, v/proj/q 1.
"""

import sys

if "/opt/trn_rl_repo" not in sys.path:
    sys.path.insert(0, "/opt/trn_rl_repo")

import numpy as np

import concourse.bass as bass
import concourse.bacc as bacc
import concourse.tile as tile
import concourse.mybir as mybir
from concourse import bass_utils

# Problem dims (hardcoded per spec)
B, C, HH, WW = 8, 256, 64, 64
N = HH * WW            # 4096
G = 8                  # groupnorm groups
GSZ = C // G           # 32 channels/group
EPS = 1e-5
P = 128                # SBUF partitions
CT = C // P            # 2 channel tiles
NCH = 512              # query-chunk width (free dim per matmul)
NNCH = N // NCH        # 8
MT = N // P            # 32 key tiles
MP = MT // 2           # 16 key-tile pairs
SCALE = 1.0 / np.sqrt(C)
ESHIFT = -4.0          # exponent shift; cancels in normalization
INV_CNT = 1.0 / (GSZ * N)

F32 = mybir.dt.float32
F32R = mybir.dt.float32r
FP8 = mybir.dt.float8e4
DR = mybir.MatmulPerfMode.DoubleRow


def _emit(tc, d, out_d):
    from contextlib import ExitStack

    nc = tc.nc
    AF = mybir.ActivationFunctionType
    OP = mybir.AluOpType
    AX = mybir.AxisListType.X
    ts, ds = bass.ts, bass.ds

    with ExitStack() as ctx:
        const = ctx.enter_context(tc.tile_pool(name="const", bufs=1))
        big = ctx.enter_context(tc.tile_pool(name="big", bufs=1))
        work = ctx.enter_context(tc.tile_pool(name="work", bufs=3))
        small = ctx.enter_context(tc.tile_pool(name="small", bufs=4))
        outp = ctx.enter_context(tc.tile_pool(name="outp", bufs=3))
        # PSUM: 8 banks total. s-pair 2 bufs x 2 banks, attn 2 banks,
        # den 1 bank, v/proj shared 1 bank.
        psS = ctx.enter_context(tc.tile_pool(name="psS", bufs=2, space="PSUM"))
        psA = ctx.enter_context(tc.tile_pool(name="psA", bufs=1, space="PSUM"))
        psD = ctx.enter_context(tc.tile_pool(name="psD", bufs=1, space="PSUM"))
        psP = ctx.enter_context(tc.tile_pool(name="psP", bufs=1, space="PSUM"))

        # ---------------- DMAs: packed consts first, then x on 4 queues -----
        # consts_a columns: b_q(2) b_k(2) b_o(2) gn_w(2) gn_b(2) fmask(2x8)
        ca = const.tile([P, 26], F32, name="ca")
        nc.scalar.dma_start(out=ca, in_=d["consts_a"])
        bm_sb = const.tile([G, CT, P], F32, name="bm_sb")
        nc.scalar.dma_start(out=bm_sb[:, :, :], in_=d["bmask"])
        BQ, BK, BO = 0, 2, 4         # ca column offsets

        # x on the sync/gpsimd queues only (the scalar queue carries the small
        # consts + fp8 weights and must stay clear for the GN squares)
        NC4 = 4                      # head chunks per channel-tile
        CW = N // NC4                # 1024 columns per chunk
        BF16 = mybir.dt.bfloat16
        xb_sb = big.tile([P, CT, N], BF16, name="xb_sb")
        x_sb = big.tile([P, CT, N], F32, name="x_sb")
        xq = [nc.sync, nc.gpsimd, nc.scalar]
        for c in range(NC4):
            for t in range(CT):
                csl = ds(c * CW, CW)
                xq[(c * CT + t) % 3].dma_start(out=xb_sb[:, t, csl],
                                               in_=d["x_bf"][ts(t, P), csl])

        # qkv weights arrive bf16 pair-packed ([cin_half, 2, cout]); the GN
        # per-channel scale is folded into them on-device -> fp8. wo is fp8.
        wqb_sb = const.tile([P, CT, C], BF16, name="wqb_sb")
        wkb_sb = const.tile([P, CT, C], BF16, name="wkb_sb")
        wvb_sb = const.tile([P, CT, C], BF16, name="wvb_sb")
        wo_sb = const.tile([P, CT, C], FP8, name="wo_sb")
        for sb, dr in ((wkb_sb, d["wkb"]), (wqb_sb, d["wqb"]),
                       (wvb_sb, d["wvb"]), (wo_sb, d["wo8"])):
            nc.scalar.dma_start(out=sb, in_=dr)
        wq_sb = const.tile([P, CT, C], FP8, name="wq_sb")
        wk_sb = const.tile([P, CT, C], FP8, name="wk_sb")
        wv_sb = const.tile([P, CT, C], FP8, name="wv_sb")

        # fp8 ones pair for the den matmul (pair-dim stride kept at 16B)
        ones_sb = const.tile([P, CT, 16], FP8, name="ones_sb")
        nc.gpsimd.memset(ones_sb, 1.0)
        zero_sb = const.tile([P, 1], F32, name="zero_sb")
        nc.gpsimd.memset(zero_sb, 0.0)
        zerob_sb = const.tile([P, 1], BF16, name="zerob_sb")
        nc.gpsimd.memset(zerob_sb, 0.0)
        esh_sb = const.tile([P, 1], F32, name="esh_sb")
        nc.gpsimd.memset(esh_sb, ESHIFT)
        eps_sb = const.tile([G, 1], F32, name="eps_sb")
        nc.gpsimd.memset(eps_sb, EPS)

        # ---------------- GroupNorm stats (pipelined behind x DMA) ----------
        # Both row-sum and x^2 row-sum run on vector (tensor_tensor_reduce),
        # keeping the scalar engine's activation-table sequence to
        # Sqrt/Identity -> Exp with both loads preloaded off the critical
        # path (dummy ops below).
        x8_sb = big.tile([P, CT, N], FP8, name="x8_sb")  # raw x cast to fp8
        sq_scr = small.tile([P, 2, CW], F32, name="sq_scr")
        stat = small.tile([P, CT, NC4, 2], F32, name="stat")
        gps = psS.tile([G, 2], F32, tag="s", name="gps")
        for c in range(NC4):
            for t in range(CT):
                csl = ds(c * CW, CW)
                nc.vector.reduce_sum(out=stat[:, t, c, 0:1], in_=xb_sb[:, t, csl],
                                     axis=AX)
                nc.scalar.activation(out=sq_scr[:, (c * CT + t) % 2, :],
                                     in_=xb_sb[:, t, csl],
                                     func=AF.Square, bias=zero_sb,
                                     accum_out=stat[:, t, c, 1:2])
                nc.vector.tensor_copy(out=x8_sb[:, t, csl], in_=xb_sb[:, t, csl])
                # PE warm-up during the head: short bf16 matmul on the chunk
                if t == 0:
                    warm = psS.tile([1, P], F32, tag="s", name="warm")
                    nc.tensor.matmul(warm, lhsT=zerob_sb,
                                     rhs=xb_sb[:, t, ds(c * CW, P)],
                                     start=True, stop=True)
            for t in range(CT):
                nc.tensor.matmul(gps, lhsT=ca[:, ds(10 + G * t, G)],
                                 rhs=stat[:, t, c, :],
                                 start=(c == 0 and t == 0),
                                 stop=(c == NC4 - 1 and t == CT - 1))
            if c == 1:
                # preload the Sqrt/Identity table in a DMA-wait gap; the
                # Square reload after it also hides in the stats pipeline
                nc.scalar.activation(out=eps_sb, in_=eps_sb, func=AF.Sqrt,
                                     bias=eps_sb)
                nc.gpsimd.memset(eps_sb, EPS)
        for w in range(8):
            warm = psS.tile([1, NCH], F32, tag="s", name="warmb")
            nc.tensor.matmul(warm, lhsT=zerob_sb,
                             rhs=xb_sb[:, w % 2, ds(((w * 3) % 7) * NCH, NCH)],
                             start=True, stop=True)
        grp = small.tile([G, 2], F32, name="grp")    # [mean, rstd]
        gtmp = small.tile([G, 3], F32, name="gtmp")
        nc.vector.tensor_scalar_mul(out=grp[:, 0:1], in0=gps[:, 0:1], scalar1=INV_CNT)
        nc.vector.tensor_scalar_mul(out=gtmp[:, 0:1], in0=gps[:, 1:2], scalar1=INV_CNT)
        nc.vector.tensor_mul(out=gtmp[:, 1:2], in0=grp[:, 0:1], in1=grp[:, 0:1])
        nc.vector.tensor_sub(out=gtmp[:, 2:3], in0=gtmp[:, 0:1], in1=gtmp[:, 1:2])
        nc.scalar.activation(out=gtmp[:, 2:3], in_=gtmp[:, 2:3], func=AF.Sqrt,
                             bias=eps_sb)
        nc.vector.reciprocal(out=grp[:, 1:2], in_=gtmp[:, 2:3])

        ab = small.tile([P, CT, 2], F32, name="ab")  # per-channel scale a, bias b
        for t in range(CT):
            cps = psS.tile([P, 2], F32, tag="s", name="cps")
            nc.tensor.matmul(cps, lhsT=bm_sb[:, t, :], rhs=grp, start=True, stop=True)
            nc.vector.tensor_mul(out=ab[:, t, 0:1], in0=cps[:, 1:2], in1=ca[:, 6 + t:7 + t])
            nc.vector.tensor_mul(out=ab[:, t, 1:2], in0=cps[:, 0:1], in1=ab[:, t, 0:1])
            nc.vector.tensor_sub(out=ab[:, t, 1:2], in0=ca[:, 8 + t:9 + t], in1=ab[:, t, 1:2])


        # ------- fold GN affine into the weights + device-side bias terms ---
        # w' = fp8(w_bf16 * a[cin]);  beta = w' @ (b_full/a) + b_host.
        # The v-path bias flows through softmax (rows sum to 1) into the
        # output: delta = Wo @ (Wv' bvec); combined with the host b_o fold.
        q_sb = big.tile([P, CT, N], FP8, name="q_sb")   # (c_half, n)
        k_sb = big.tile([P, CT, N], FP8, name="k_sb")
        vT_sb = big.tile([P, MT, C], FP8, name="vT_sb")  # (n, c), n on partitions

        rab = small.tile([P, CT, 2], F32, name="rab")   # [1/a, b_full/a]
        nc.vector.reciprocal(out=rab[:, :, 0:1], in_=ab[:, :, 0:1])
        nc.vector.tensor_mul(out=rab[:, :, 1:2], in0=ab[:, :, 1:2], in1=rab[:, :, 0:1])
        bvec8 = small.tile([P, CT, 16], FP8, name="bvec8")
        nc.vector.tensor_copy(out=bvec8[:, :, 0:1], in_=rab[:, :, 1:2])

        for j, (w8, wb) in enumerate(((wk_sb, wkb_sb), (wq_sb, wqb_sb),
                                      (wv_sb, wvb_sb))):
            for t in range(CT):
                if (j + t) % 2 == 0:
                    nc.vector.tensor_scalar_mul(out=w8[:, t, :], in0=wb[:, t, :],
                                                scalar1=ab[:, t, 0:1])
                else:
                    nc.scalar.activation(out=w8[:, t, :], in_=wb[:, t, :],
                                         func=AF.Identity, bias=zero_sb,
                                         scale=ab[:, t, 0:1])

        # tiny DR matmuls for the bias terms, all into one psP bank
        bps = psP.tile([P, NCH], F32, tag="proj", name="bps")
        for i, w8 in enumerate((wq_sb, wk_sb)):
            for tq in range(CT):
                nc.tensor.matmul(bps[:, 2 * i + tq:2 * i + tq + 1],
                                 lhsT=w8[:, :, ts(tq, P)], rhs=bvec8[:, :, 0:1],
                                 start=True, stop=True, perf_mode=DR)
        beta = small.tile([P, 6], F32, name="beta")     # [q0 q1 k0 k1 o0 o1]
        nc.vector.tensor_add(out=beta[:, 0:4], in0=bps[:, 0:4], in1=ca[:, 0:4])

        def _vbias():
            for tq in range(CT):
                nc.tensor.matmul(bps[:, 4 + tq:5 + tq],
                                 lhsT=wv_sb[:, :, ts(tq, P)], rhs=bvec8[:, :, 0:1],
                                 start=True, stop=True, perf_mode=DR)
            usb8 = small.tile([P, CT, 16], FP8, name="usb8")
            for t in range(CT):
                nc.vector.tensor_copy(out=usb8[:, t, 0:1], in_=bps[:, 4 + t:5 + t])
            for co in range(CT):
                nc.tensor.matmul(bps[:, 6 + co:7 + co],
                                 lhsT=wo_sb[:, :, ts(co, P)], rhs=usb8[:, :, 0:1],
                                 start=True, stop=True, perf_mode=DR)
            nc.vector.tensor_add(out=beta[:, 4:6], in0=bps[:, 6:8], in1=ca[:, 4:6])

        # K for all slices now; Q is produced one nch ahead inside the body
        # (on the shared psP bank) — only q(nch 0) is made in the head.
        for s in range(NNCH):
            nsl = ds(s * NCH, NCH)
            # alternate the production tile between the psS slots and the
            # (still idle) attn bank pair for a deeper head pipeline
            pool, tag = (psS, "s") if s % 2 == 0 else (psA, "attn")
            pp = pool.tile([P, CT, NCH], F32, tag=tag, name="ppqk")
            for tq in range(CT):
                nc.tensor.matmul(pp[:, tq, :],
                                 lhsT=wk_sb[:, :, ts(tq, P)],
                                 rhs=x8_sb[:, :, nsl],
                                 start=True, stop=True, perf_mode=DR)
            for tq in range(CT):
                eng = nc.vector if (s + tq) % 2 == 0 else nc.scalar
                if eng is nc.vector:
                    nc.vector.tensor_scalar_add(
                        out=k_sb[:, tq, nsl], in0=pp[:, tq, :],
                        scalar1=beta[:, 2 + tq:3 + tq])
                else:
                    nc.scalar.activation(
                        out=k_sb[:, tq, nsl], in_=pp[:, tq, :],
                        func=AF.Identity, bias=beta[:, 2 + tq:3 + tq])

        # full-precision x for the residual: streams in under the body on the
        # sync queue. The DMA engines pull queued descriptors immediately and
        # would steal HBM bandwidth from the head, so gate each chunk behind
        # k-production via a WAW dependency on one element per chunk region.
        nc.gpsimd.tensor_copy(out=x_sb[0:1, :, 0:N:CW], in_=k_sb[0:1, :, 0:N:CW])
        for c in range(NC4):
            for t in range(CT):
                csl = ds(c * CW, CW)
                nc.sync.dma_start(out=x_sb[:, t, csl], in_=d["x"][ts(t, P), csl])

        def _qprod(s, tq, eng):
            nsl = ds(s * NCH, NCH)
            pq = psP.tile([P, NCH], F32, tag="proj", name="pq")
            nc.tensor.matmul(pq, lhsT=wq_sb[:, :, ts(tq, P)],
                             rhs=x8_sb[:, :, nsl],
                             start=True, stop=True, perf_mode=DR)
            eng.tensor_scalar_add(out=q_sb[:, tq, nsl], in0=pq,
                                  scalar1=beta[:, tq:tq + 1])

        pp = psS.tile([P, CT, NCH], F32, tag="s", name="ppq0")
        for tq in range(CT):
            nc.tensor.matmul(pp[:, tq, :], lhsT=wq_sb[:, :, ts(tq, P)],
                             rhs=x8_sb[:, :, 0:NCH],
                             start=True, stop=True, perf_mode=DR)
        for tq in range(CT):
            eng = nc.vector if tq == 0 else nc.scalar
            if tq == 0:
                nc.vector.tensor_scalar_add(out=q_sb[:, tq, 0:NCH],
                                            in0=pp[:, tq, :],
                                            scalar1=beta[:, tq:tq + 1])
            else:
                nc.scalar.activation(out=q_sb[:, tq, 0:NCH], in_=pp[:, tq, :],
                                     func=AF.Identity, bias=beta[:, tq:tq + 1])
        _vbias()
        # preload the Exp activation table before the body needs it
        nc.scalar.activation(out=gtmp[:, 0:1], in_=eps_sb, func=AF.Exp,
                             bias=eps_sb)

        # ---------------- attention + output projection (fp8 DoubleRow) -----
        # v-tile pairs are produced inside nch 0's stream (PSUM bank shared
        # with proj; b_v is folded into b_out host-side).
        # Flat (nch, mp) stream; AV/den trail the scores/exp stream by LAG
        # units ACROSS nch boundaries so the per-nch PSUM handoff (attn CAST,
        # den copy) hides under the next nch's score matmuls.
        LAG = 4
        attns, dens = {}, {}

        tstate = {}

        def _tailA(nch):
            # atts copy FIRST (per channel-half, so half 0 can start as soon
            # as its last AV lands): it releases the attn PSUM pair the next
            # nch's AV matmuls are waiting on. For the final nch the den
            # chain is more urgent than the PSUM release.
            atts = outp.tile([P, CT, NCH], FP8, tag="att", name="atts", bufs=3)
            attp = attns.pop(nch)

            def _cast():
                nc.vector.tensor_copy(out=atts, in_=attp)

            def _den():
                den_sb = small.tile([1, NCH], F32, tag="den_sb", name="den_sb", bufs=2)
                nc.vector.tensor_copy(out=den_sb, in_=dens.pop(nch))
                rden = small.tile([1, NCH], F32, tag="rden", name="rden", bufs=2)
                nc.vector.reciprocal_approx_fast(out=rden, in_=den_sb)
                rdenb = outp.tile([P, NCH], F32, tag="rdenb", name="rdenb", bufs=2)
                nc.gpsimd.partition_broadcast(rdenb, rden)
                return rdenb

            if nch == NNCH - 1:
                rdenb = _den()
                _cast()
            else:
                _cast()
                rdenb = _den()
            tstate[nch] = (atts, rdenb)

        def _tailB(nch):
            nsl = ds(nch * NCH, NCH)
            atts, rdenb = tstate.pop(nch)
            last = nch == NNCH - 1
            # proj per output-channel half on a single PSUM bank; copy to SBUF
            # immediately (PSUM release must not be gated on the rden chain).
            # The final nch has no release pressure: read the PSUM directly
            # and split its stores into halves on both DMA queues.
            for co in range(CT):
                pj = psP.tile([P, NCH], F32, tag="proj", name="pj")
                nc.tensor.matmul(pj, lhsT=wo_sb[:, :, ts(co, P)], rhs=atts,
                                 start=True, stop=True, perf_mode=DR)
                if last:
                    src_pj = pj
                else:
                    src_pj = outp.tile([P, NCH], F32, tag="pjs", name="pjs", bufs=3)
                    nc.vector.tensor_copy(out=src_pj, in_=pj)
                f = outp.tile([P, NCH], F32, tag="fout", name="f", bufs=3)
                nc.vector.tensor_tensor(out=f, in0=src_pj, in1=rdenb, op=OP.mult)
                nc.vector.scalar_tensor_tensor(out=f, in0=f, scalar=beta[:, 4 + co:5 + co],
                                               in1=x_sb[:, co, nsl],
                                               op0=OP.add, op1=OP.add)
                if last:
                    hw = NCH // 2
                    nc.sync.dma_start(out=out_d[ts(co, P), ds(nch * NCH, hw)],
                                      in_=f[:, 0:hw])
                    nc.gpsimd.dma_start(out=out_d[ts(co, P), ds(nch * NCH + hw, hw)],
                                        in_=f[:, hw:NCH])
                else:
                    eng = nc.sync if co == 0 else nc.gpsimd
                    eng.dma_start(out=out_d[ts(co, P), nsl], in_=f)

        def _av(nch, mp, e):
            if mp == 0:
                attns[nch] = psA.tile([P, CT, NCH], F32, tag="attn", name="attn")
                dens[nch] = psD.tile([1, NCH], F32, tag="den", name="den")
            for ch in range(CT):
                nc.tensor.matmul(attns[nch][:, ch, :],
                                 lhsT=vT_sb[:, ds(2 * mp, 2), ts(ch, P)],
                                 rhs=e,
                                 start=(mp == 0), stop=(mp == MP - 1),
                                 perf_mode=DR)
            nc.tensor.matmul(dens[nch], lhsT=ones_sb[:, :, 0:1], rhs=e,
                             start=(mp == 0), stop=(mp == MP - 1),
                             perf_mode=DR)

        def _pop(unit):
            if unit[0] == "tailA":
                _tailA(unit[1])
            elif unit[0] == "tailB":
                _tailB(unit[1])
            else:
                _av(*unit[1:])

        pend = []
        for nch in range(NNCH):
            nsl = ds(nch * NCH, NCH)
            for mp in range(MP):
                if nch == 0:
                    vt = psP.tile([P, CT, C], F32, tag="proj", name="vt")
                    for half in range(2):
                        nc.tensor.matmul(vt[:, half, :],
                                         lhsT=x8_sb[:, :, ts(2 * mp + half, P)],
                                         rhs=wv_sb,
                                         start=True, stop=True, perf_mode=DR)
                    nc.vector.tensor_copy(out=vT_sb[:, ds(2 * mp, 2), :], in_=vt)
                if nch + 1 < NNCH and mp in (10, 11):
                    _qprod(nch + 1, mp - 10, nc.vector)
                sp = psS.tile([P, CT, NCH], F32, tag="s", name="sp")
                for half in range(2):
                    nc.tensor.matmul(sp[:, half, :],
                                     lhsT=k_sb[:, :, ts(2 * mp + half, P)],
                                     rhs=q_sb[:, :, nsl],
                                     start=True, stop=True, perf_mode=DR)
                e = work.tile([P, 2, NCH], FP8, tag="e", name="e", bufs=8)
                nc.scalar.activation(out=e, in_=sp, func=AF.Exp,
                                     bias=esh_sb, scale=SCALE)
                pend.append(("av", nch, mp, e))
                while len(pend) > LAG:
                    _pop(pend.pop(0))
                if mp == MP - 1:
                    pend.append(("tailA", nch))
                    pend.append(("tailB", nch))
        for unit in pend:
            _pop(unit)


def build_program():
    nc = bacc.Bacc("TRN2", target_bir_lowering=False, debug=False, num_devices=B)
    d = {}

    def din(name, shape, dt_=F32):
        d[name] = nc.dram_tensor(name, list(shape), dt_, kind="ExternalInput").ap()

    din("x", (C, N))
    din("x_bf", (C, N), mybir.dt.bfloat16)
    din("wqb", (P, CT, C), mybir.dt.bfloat16)
    din("wkb", (P, CT, C), mybir.dt.bfloat16)
    din("wvb", (P, CT, C), mybir.dt.bfloat16)
    din("wo8", (P, CT, C), FP8)
    din("consts_a", (P, 26))
    din("bmask", (G, CT * P))
    out_d = nc.dram_tensor("out", [C, N], F32, kind="ExternalOutput").ap()

    with tile.TileContext(nc) as tc:
        _emit(tc, d, out_d)
    nc.compile()
    return nc


_PROG = None


def _get_program():
    global _PROG
    if _PROG is None:
        _PROG = build_program()
    return _PROG


def make_in_maps(inputs):
    x = np.ascontiguousarray(np.asarray(inputs["x"], dtype=np.float32))
    w_qkv = np.asarray(inputs["w_qkv"], dtype=np.float32)
    b_qkv = np.asarray(inputs["b_qkv"], dtype=np.float32)
    w_out = np.asarray(inputs["w_out"], dtype=np.float32)
    b_out = np.asarray(inputs["b_out"], dtype=np.float32)
    gn_scale = np.asarray(inputs["gn_scale"], dtype=np.float32)
    gn_bias = np.asarray(inputs["gn_bias"], dtype=np.float32)

    fmask = np.zeros((CT, P, G), dtype=np.float32)
    for t in range(CT):
        for p in range(P):
            fmask[t, p, (t * P + p) // GSZ] = 1.0
    # bmask[g, t*P+p] = fmask[t, p, g]
    bmask = np.ascontiguousarray(fmask.transpose(2, 0, 1).reshape(G, CT * P))

    consts_a = np.zeros((P, 26), dtype=np.float32)
    bo_eff = b_out + w_out @ b_qkv[2 * C:3 * C]   # b_v folded (softmax sums to 1)
    for t in range(CT):
        rows = slice(t * P, (t + 1) * P)
        consts_a[:, 0 + t] = b_qkv[0:C][rows]
        consts_a[:, 2 + t] = b_qkv[C:2 * C][rows]
        consts_a[:, 4 + t] = bo_eff[rows]
        consts_a[:, 6 + t] = gn_scale[rows]
        consts_a[:, 8 + t] = gn_bias[rows]
        consts_a[:, 10 + G * t:10 + G * (t + 1)] = fmask[t]

    import ml_dtypes
    E4 = ml_dtypes.float8_e4m3
    BF = ml_dtypes.bfloat16

    def pack(w, dt):
        # [cout, cin] -> lhsT/rhs pair layout [cin_half, 2, cout]
        return np.ascontiguousarray(
            w.T.reshape(CT, P, C).transpose(1, 0, 2)).astype(dt)

    common = {
        "wqb": pack(w_qkv[0:C], BF),
        "wkb": pack(w_qkv[C:2 * C], BF),
        "wvb": pack(w_qkv[2 * C:3 * C], BF),
        "wo8": pack(w_out, E4),
        "consts_a": consts_a,
        "bmask": bmask,
    }
    return [dict(common, x=np.ascontiguousarray(x[b].reshape(C, N)),
                 x_bf=np.ascontiguousarray(x[b].reshape(C, N).astype(BF)))
            for b in range(B)]


def run(inputs, trace=False):
    nc = _get_program()
    in_maps = make_in_maps(inputs)
    res = bass_utils.run_bass_kernel_spmd(nc, in_maps, core_ids=list(range(B)),
                                          trace=trace)
    out = np.stack([res.results[b]["out"] for b in range(B)])
    return out.reshape(B, C, HH, WW), res


def kernel(**inputs):
    out, _ = run(inputs, trace=False)
    return out


# revision 75
# speedup vs baseline: 1.0048x; 1.0029x over previous
"""AttentionBlock (GroupNorm + single-head NxN attention + residual) on 8 TRN2 cores.

Data-parallel: batch dim (B=8) sharded 1 image per NeuronCore. Per core:

  x (C=256, N=4096) f32 -> GroupNorm stats (vector row-sums + scalar
  square-accum, pipelined behind the x DMA); the GN affine is FOLDED INTO
  THE QKV WEIGHTS: w' = fp8(w_bf16 * a[cin]) with a = rstd*gn_scale, and
  the mean/bias offset is restored through tiny on-device bias matmuls
  (beta = w' @ (b_full/a) + b_host; the v-path bias passes through softmax
  and lands in the output bias). x is cast to raw fp8 once.

  All heavy matmuls are fp8e4m3 DoubleRow (contraction 256 = full C, or an
  m-pair of two 128-key tiles, in ONE matmul at 2 MACs/cell/cycle):
    k = Wk' x8, q = Wq' x8 (q produced one nch ahead, inside the body)
    s = k^T q  -> e = exp(s/16 - 4) fp8 (one paired exp per two key tiles;
    the scalar engine runs Exp back-to-back and is the body bottleneck)
    attn_u = v @ e, den = ones @ e, proj_u = Wo @ attn_u
    out = proj_u * (1/den) + b_out_eff + x

  The exponent shift -4 keeps exp under the fp8 max (240) and cancels in
  the normalization. Softmax rows sum to 1, so v/out biases fold exactly.

Schedule: a flat (nch, key-pair) software pipeline; AV/den matmuls and the
per-nch tail (atts cast, reciprocal, projection, residual, store) trail the
scores/exp stream via a deferral queue so the in-order PE never waits on
the PSUM handoffs. PSUM: scores-pair 2x2 banks, attn 2, den 1, v/proj/q 1.
"""

import sys

if "/opt/trn_rl_repo" not in sys.path:
    sys.path.insert(0, "/opt/trn_rl_repo")

import numpy as np

import concourse.bass as bass
import concourse.bacc as bacc
import concourse.tile as tile
import concourse.mybir as mybir
from concourse import bass_utils

# Problem dims (hardcoded per spec)
B, C, HH, WW = 8, 256, 64, 64
N = HH * WW            # 4096
G = 8                  # groupnorm groups
GSZ = C // G           # 32 channels/group
EPS = 1e-5
P = 128                # SBUF partitions
CT = C // P            # 2 channel tiles
NCH = 512              # query-chunk width (free dim per matmul)
NNCH = N // NCH        # 8
MT = N // P            # 32 key tiles
MP = MT // 2           # 16 key-tile pairs
SCALE = 1.0 / np.sqrt(C)
ESHIFT = -4.0          # exponent shift; cancels in normalization
INV_CNT = 1.0 / (GSZ * N)

F32 = mybir.dt.float32
F32R = mybir.dt.float32r
FP8 = mybir.dt.float8e4
DR = mybir.MatmulPerfMode.DoubleRow


def _emit(tc, d, out_d):
    from contextlib import ExitStack

    nc = tc.nc
    AF = mybir.ActivationFunctionType
    OP = mybir.AluOpType
    AX = mybir.AxisListType.X
    ts, ds = bass.ts, bass.ds

    with ExitStack() as ctx:
        const = ctx.enter_context(tc.tile_pool(name="const", bufs=1))
        big = ctx.enter_context(tc.tile_pool(name="big", bufs=1))
        work = ctx.enter_context(tc.tile_pool(name="work", bufs=3))
        small = ctx.enter_context(tc.tile_pool(name="small", bufs=4))
        outp = ctx.enter_context(tc.tile_pool(name="outp", bufs=3))
        # PSUM: 8 banks total. s-pair 2 bufs x 2 banks, attn 2 banks,
        # den 1 bank, v/proj shared 1 bank.
        psS = ctx.enter_context(tc.tile_pool(name="psS", bufs=2, space="PSUM"))
        psA = ctx.enter_context(tc.tile_pool(name="psA", bufs=1, space="PSUM"))
        psD = ctx.enter_context(tc.tile_pool(name="psD", bufs=1, space="PSUM"))
        psP = ctx.enter_context(tc.tile_pool(name="psP", bufs=1, space="PSUM"))

        # ---------------- DMAs: packed consts first, then x on 4 queues -----
        # consts_a columns: b_q(2) b_k(2) b_o(2) gn_w(2) gn_b(2) fmask(2x8)
        ca = const.tile([P, 26], F32, name="ca")
        nc.scalar.dma_start(out=ca, in_=d["consts_a"])
        bm_sb = const.tile([G, CT, P], F32, name="bm_sb")
        nc.scalar.dma_start(out=bm_sb[:, :, :], in_=d["bmask"])
        BQ, BK, BO = 0, 2, 4         # ca column offsets

        # x on the sync/gpsimd queues only (the scalar queue carries the small
        # consts + fp8 weights and must stay clear for the GN squares)
        NC4 = 4                      # head chunks per channel-tile
        CW = N // NC4                # 1024 columns per chunk
        BF16 = mybir.dt.bfloat16
        xb_sb = big.tile([P, CT, N], BF16, name="xb_sb")
        x_sb = big.tile([P, CT, N], F32, name="x_sb")
        xq = [nc.sync, nc.gpsimd, nc.scalar]
        for c in range(NC4):
            for t in range(CT):
                csl = ds(c * CW, CW)
                xq[(c * CT + t) % 3].dma_start(out=xb_sb[:, t, csl],
                                               in_=d["x_bf"][ts(t, P), csl])

        # qkv weights arrive bf16 pair-packed ([cin_half, 2, cout]); the GN
        # per-channel scale is folded into them on-device -> fp8. wo is fp8.
        wqb_sb = const.tile([P, CT, C], BF16, name="wqb_sb")
        wkb_sb = const.tile([P, CT, C], BF16, name="wkb_sb")
        wvb_sb = const.tile([P, CT, C], BF16, name="wvb_sb")
        wo_sb = const.tile([P, CT, C], FP8, name="wo_sb")
        for sb, dr in ((wkb_sb, d["wkb"]), (wqb_sb, d["wqb"]),
                       (wvb_sb, d["wvb"]), (wo_sb, d["wo8"])):
            nc.scalar.dma_start(out=sb, in_=dr)
        wq_sb = const.tile([P, CT, C], FP8, name="wq_sb")
        wk_sb = const.tile([P, CT, C], FP8, name="wk_sb")
        wv_sb = const.tile([P, CT, C], FP8, name="wv_sb")

        # fp8 ones pair for the den matmul (pair-dim stride kept at 16B)
        ones_sb = const.tile([P, CT, 16], FP8, name="ones_sb")
        nc.gpsimd.memset(ones_sb, 1.0)
        zero_sb = const.tile([P, 1], F32, name="zero_sb")
        nc.gpsimd.memset(zero_sb, 0.0)
        zerob_sb = const.tile([P, 1], BF16, name="zerob_sb")
        nc.gpsimd.memset(zerob_sb, 0.0)
        esh_sb = const.tile([P, 1], F32, name="esh_sb")
        nc.gpsimd.memset(esh_sb, ESHIFT)
        eps_sb = const.tile([G, 1], F32, name="eps_sb")
        nc.gpsimd.memset(eps_sb, EPS)

        # ---------------- GroupNorm stats (pipelined behind x DMA) ----------
        # Both row-sum and x^2 row-sum run on vector (tensor_tensor_reduce),
        # keeping the scalar engine's activation-table sequence to
        # Sqrt/Identity -> Exp with both loads preloaded off the critical
        # path (dummy ops below).
        x8_sb = big.tile([P, CT, N], FP8, name="x8_sb")  # raw x cast to fp8
        sq_scr = small.tile([P, 2, CW], F32, name="sq_scr")
        stat = small.tile([P, CT, NC4, 2], F32, name="stat")
        gps = psS.tile([G, 2], F32, tag="s", name="gps")
        for c in range(NC4):
            for t in range(CT):
                csl = ds(c * CW, CW)
                nc.vector.reduce_sum(out=stat[:, t, c, 0:1], in_=xb_sb[:, t, csl],
                                     axis=AX)
                nc.scalar.activation(out=sq_scr[:, (c * CT + t) % 2, :],
                                     in_=xb_sb[:, t, csl],
                                     func=AF.Square, bias=zero_sb,
                                     accum_out=stat[:, t, c, 1:2])
                nc.vector.tensor_copy(out=x8_sb[:, t, csl], in_=xb_sb[:, t, csl])
                # PE warm-up during the head: short bf16 matmul on the chunk
                if t == 0:
                    warm = psS.tile([1, P], F32, tag="s", name="warm")
                    nc.tensor.matmul(warm, lhsT=zerob_sb,
                                     rhs=xb_sb[:, t, ds(c * CW, P)],
                                     start=True, stop=True)
            for t in range(CT):
                nc.tensor.matmul(gps, lhsT=ca[:, ds(10 + G * t, G)],
                                 rhs=stat[:, t, c, :],
                                 start=(c == 0 and t == 0),
                                 stop=(c == NC4 - 1 and t == CT - 1))
            if c == 1:
                # preload the Sqrt/Identity table in a DMA-wait gap; the
                # Square reload after it also hides in the stats pipeline
                nc.scalar.activation(out=eps_sb, in_=eps_sb, func=AF.Sqrt,
                                     bias=eps_sb)
                nc.gpsimd.memset(eps_sb, EPS)
        for w in range(8):
            warm = psS.tile([1, NCH], F32, tag="s", name="warmb")
            nc.tensor.matmul(warm, lhsT=zerob_sb,
                             rhs=xb_sb[:, w % 2, ds(((w * 3) % 7) * NCH, NCH)],
                             start=True, stop=True)
        grp = small.tile([G, 2], F32, name="grp")    # [mean, rstd]
        gtmp = small.tile([G, 3], F32, name="gtmp")
        nc.vector.tensor_scalar_mul(out=grp[:, 0:1], in0=gps[:, 0:1], scalar1=INV_CNT)
        nc.vector.tensor_scalar_mul(out=gtmp[:, 0:1], in0=gps[:, 1:2], scalar1=INV_CNT)
        nc.vector.tensor_mul(out=gtmp[:, 1:2], in0=grp[:, 0:1], in1=grp[:, 0:1])
        nc.vector.tensor_sub(out=gtmp[:, 2:3], in0=gtmp[:, 0:1], in1=gtmp[:, 1:2])
        nc.scalar.activation(out=gtmp[:, 2:3], in_=gtmp[:, 2:3], func=AF.Sqrt,
                             bias=eps_sb)
        nc.vector.reciprocal(out=grp[:, 1:2], in_=gtmp[:, 2:3])

        ab = small.tile([P, CT, 2], F32, name="ab")  # per-channel scale a, bias b
        for t in range(CT):
            cps = psS.tile([P, 2], F32, tag="s", name="cps")
            nc.tensor.matmul(cps, lhsT=bm_sb[:, t, :], rhs=grp, start=True, stop=True)
            nc.vector.tensor_mul(out=ab[:, t, 0:1], in0=cps[:, 1:2], in1=ca[:, 6 + t:7 + t])
            nc.vector.tensor_mul(out=ab[:, t, 1:2], in0=cps[:, 0:1], in1=ab[:, t, 0:1])
            nc.vector.tensor_sub(out=ab[:, t, 1:2], in0=ca[:, 8 + t:9 + t], in1=ab[:, t, 1:2])


        # ------- fold GN affine into the weights + device-side bias terms ---
        # w' = fp8(w_bf16 * a[cin]);  beta = w' @ (b_full/a) + b_host.
        # The v-path bias flows through softmax (rows sum to 1) into the
        # output: delta = Wo @ (Wv' bvec); combined with the host b_o fold.
        q_sb = big.tile([P, CT, N], FP8, name="q_sb")   # (c_half, n)
        k_sb = big.tile([P, CT, N], FP8, name="k_sb")
        vT_sb = big.tile([P, MT, C], FP8, name="vT_sb")  # (n, c), n on partitions

        rab = small.tile([P, CT, 2], F32, name="rab")   # [1/a, b_full/a]
        nc.vector.reciprocal(out=rab[:, :, 0:1], in_=ab[:, :, 0:1])
        nc.vector.tensor_mul(out=rab[:, :, 1:2], in0=ab[:, :, 1:2], in1=rab[:, :, 0:1])
        bvec8 = small.tile([P, CT, 16], FP8, name="bvec8")
        nc.vector.tensor_copy(out=bvec8[:, :, 0:1], in_=rab[:, :, 1:2])

        for j, (w8, wb) in enumerate(((wk_sb, wkb_sb), (wq_sb, wqb_sb),
                                      (wv_sb, wvb_sb))):
            for t in range(CT):
                if (j + t) % 2 == 0:
                    nc.vector.tensor_scalar_mul(out=w8[:, t, :], in0=wb[:, t, :],
                                                scalar1=ab[:, t, 0:1])
                else:
                    nc.scalar.activation(out=w8[:, t, :], in_=wb[:, t, :],
                                         func=AF.Identity, bias=zero_sb,
                                         scale=ab[:, t, 0:1])

        # tiny DR matmuls for the bias terms, all into one psP bank
        bps = psP.tile([P, NCH], F32, tag="proj", name="bps")
        for i, w8 in enumerate((wq_sb, wk_sb)):
            for tq in range(CT):
                nc.tensor.matmul(bps[:, 2 * i + tq:2 * i + tq + 1],
                                 lhsT=w8[:, :, ts(tq, P)], rhs=bvec8[:, :, 0:1],
                                 start=True, stop=True, perf_mode=DR)
        beta = small.tile([P, 6], F32, name="beta")     # [q0 q1 k0 k1 o0 o1]
        nc.vector.tensor_add(out=beta[:, 0:4], in0=bps[:, 0:4], in1=ca[:, 0:4])

        def _vbias():
            for tq in range(CT):
                nc.tensor.matmul(bps[:, 4 + tq:5 + tq],
                                 lhsT=wv_sb[:, :, ts(tq, P)], rhs=bvec8[:, :, 0:1],
                                 start=True, stop=True, perf_mode=DR)
            usb8 = small.tile([P, CT, 16], FP8, name="usb8")
            for t in range(CT):
                nc.vector.tensor_copy(out=usb8[:, t, 0:1], in_=bps[:, 4 + t:5 + t])
            for co in range(CT):
                nc.tensor.matmul(bps[:, 6 + co:7 + co],
                                 lhsT=wo_sb[:, :, ts(co, P)], rhs=usb8[:, :, 0:1],
                                 start=True, stop=True, perf_mode=DR)
            nc.vector.tensor_add(out=beta[:, 4:6], in0=bps[:, 6:8], in1=ca[:, 4:6])

        # K for all slices now; Q is produced one nch ahead inside the body
        # (on the shared psP bank) — only q(nch 0) is made in the head.
        for s in range(NNCH):
            nsl = ds(s * NCH, NCH)
            # alternate the production tile between the psS slots and the
            # (still idle) attn bank pair for a deeper head pipeline
            pool, tag = (psS, "s") if s % 2 == 0 else (psA, "attn")
            pp = pool.tile([P, CT, NCH], F32, tag=tag, name="ppqk")
            for tq in range(CT):
                nc.tensor.matmul(pp[:, tq, :],
                                 lhsT=wk_sb[:, :, ts(tq, P)],
                                 rhs=x8_sb[:, :, nsl],
                                 start=True, stop=True, perf_mode=DR)
            for tq in range(CT):
                eng = nc.vector if (s + tq) % 2 == 0 else nc.scalar
                if eng is nc.vector:
                    nc.vector.tensor_scalar_add(
                        out=k_sb[:, tq, nsl], in0=pp[:, tq, :],
                        scalar1=beta[:, 2 + tq:3 + tq])
                else:
                    nc.scalar.activation(
                        out=k_sb[:, tq, nsl], in_=pp[:, tq, :],
                        func=AF.Identity, bias=beta[:, 2 + tq:3 + tq])

        # full-precision x for the residual: streams in under the body on the
        # sync queue. The DMA engines pull queued descriptors immediately and
        # would steal HBM bandwidth from the head, so gate each chunk behind
        # k-production via a WAW dependency on one element per chunk region.
        nc.gpsimd.tensor_copy(out=x_sb[0:1, :, 0:N:CW], in_=k_sb[0:1, :, 0:N:CW])
        for c in range(NC4):
            for t in range(CT):
                csl = ds(c * CW, CW)
                nc.sync.dma_start(out=x_sb[:, t, csl], in_=d["x"][ts(t, P), csl])

        def _qprod(s, tq, eng):
            nsl = ds(s * NCH, NCH)
            pq = psP.tile([P, NCH], F32, tag="proj", name="pq")
            nc.tensor.matmul(pq, lhsT=wq_sb[:, :, ts(tq, P)],
                             rhs=x8_sb[:, :, nsl],
                             start=True, stop=True, perf_mode=DR)
            eng.tensor_scalar_add(out=q_sb[:, tq, nsl], in0=pq,
                                  scalar1=beta[:, tq:tq + 1])

        pp = psS.tile([P, CT, NCH], F32, tag="s", name="ppq0")
        for tq in range(CT):
            nc.tensor.matmul(pp[:, tq, :], lhsT=wq_sb[:, :, ts(tq, P)],
                             rhs=x8_sb[:, :, 0:NCH],
                             start=True, stop=True, perf_mode=DR)
        for tq in range(CT):
            eng = nc.vector if tq == 0 else nc.scalar
            if tq == 0:
                nc.vector.tensor_scalar_add(out=q_sb[:, tq, 0:NCH],
                                            in0=pp[:, tq, :],
                                            scalar1=beta[:, tq:tq + 1])
            else:
                nc.scalar.activation(out=q_sb[:, tq, 0:NCH], in_=pp[:, tq, :],
                                     func=AF.Identity, bias=beta[:, tq:tq + 1])
        _vbias()
        # preload the Exp activation table before the body needs it
        nc.scalar.activation(out=gtmp[:, 0:1], in_=eps_sb, func=AF.Exp,
                             bias=eps_sb)

        # ---------------- attention + output projection (fp8 DoubleRow) -----
        # v-tile pairs are produced inside nch 0's stream (PSUM bank shared
        # with proj; b_v is folded into b_out host-side).
        # Flat (nch, mp) stream; AV/den trail the scores/exp stream by LAG
        # units ACROSS nch boundaries so the per-nch PSUM handoff (attn CAST,
        # den copy) hides under the next nch's score matmuls.
        LAG = 4
        attns, dens = {}, {}

        tstate = {}

        def _tailA(nch):
            # atts copy FIRST (per channel-half, so half 0 can start as soon
            # as its last AV lands): it releases the attn PSUM pair the next
            # nch's AV matmuls are waiting on. For the final nch the den
            # chain is more urgent than the PSUM release.
            atts = outp.tile([P, CT, NCH], FP8, tag="att", name="atts", bufs=3)
            attp = attns.pop(nch)

            def _cast():
                nc.vector.tensor_copy(out=atts, in_=attp)

            def _den():
                den_sb = small.tile([1, NCH], F32, tag="den_sb", name="den_sb", bufs=2)
                nc.vector.tensor_copy(out=den_sb, in_=dens.pop(nch))
                rden = small.tile([1, NCH], F32, tag="rden", name="rden", bufs=2)
                nc.vector.reciprocal_approx_fast(out=rden, in_=den_sb)
                rdenb = outp.tile([P, NCH], F32, tag="rdenb", name="rdenb", bufs=2)
                nc.gpsimd.partition_broadcast(rdenb, rden)
                return rdenb

            if nch == NNCH - 1:
                rdenb = _den()
                _cast()
            else:
                _cast()
                rdenb = _den()
            tstate[nch] = (atts, rdenb)

        def _tailB(nch):
            nsl = ds(nch * NCH, NCH)
            atts, rdenb = tstate.pop(nch)
            # proj per output-channel half on a single PSUM bank; copy to SBUF
            # immediately (PSUM release must not be gated on the rden chain)
            for co in range(CT):
                pj = psP.tile([P, NCH], F32, tag="proj", name="pj")
                nc.tensor.matmul(pj, lhsT=wo_sb[:, :, ts(co, P)], rhs=atts,
                                 start=True, stop=True, perf_mode=DR)
                pjs = outp.tile([P, NCH], F32, tag="pjs", name="pjs", bufs=3)
                nc.vector.tensor_copy(out=pjs, in_=pj)
                f = outp.tile([P, NCH], F32, tag="fout", name="f", bufs=3)
                nc.vector.tensor_tensor(out=f, in0=pjs, in1=rdenb, op=OP.mult)
                nc.vector.scalar_tensor_tensor(out=f, in0=f, scalar=beta[:, 4 + co:5 + co],
                                               in1=x_sb[:, co, nsl],
                                               op0=OP.add, op1=OP.add)
                eng = nc.sync if co == 0 else nc.gpsimd
                eng.dma_start(out=out_d[ts(co, P), nsl], in_=f)

        def _av(nch, mp, e):
            if mp == 0:
                attns[nch] = psA.tile([P, CT, NCH], F32, tag="attn", name="attn")
                dens[nch] = psD.tile([1, NCH], F32, tag="den", name="den")
            for ch in range(CT):
                nc.tensor.matmul(attns[nch][:, ch, :],
                                 lhsT=vT_sb[:, ds(2 * mp, 2), ts(ch, P)],
                                 rhs=e,
                                 start=(mp == 0), stop=(mp == MP - 1),
                                 perf_mode=DR)
            nc.tensor.matmul(dens[nch], lhsT=ones_sb[:, :, 0:1], rhs=e,
                             start=(mp == 0), stop=(mp == MP - 1),
                             perf_mode=DR)

        def _pop(unit):
            if unit[0] == "tailA":
                _tailA(unit[1])
            elif unit[0] == "tailB":
                _tailB(unit[1])
            else:
                _av(*unit[1:])

        pend = []
        for nch in range(NNCH):
            nsl = ds(nch * NCH, NCH)
            for mp in range(MP):
                if nch == 0:
                    vt = psP.tile([P, CT, C], F32, tag="proj", name="vt")
                    for half in range(2):
                        nc.tensor.matmul(vt[:, half, :],
                                         lhsT=x8_sb[:, :, ts(2 * mp + half, P)],
                                         rhs=wv_sb,
                                         start=True, stop=True, perf_mode=DR)
                    nc.vector.tensor_copy(out=vT_sb[:, ds(2 * mp, 2), :], in_=vt)
                if nch + 1 < NNCH and mp in (10, 11):
                    _qprod(nch + 1, mp - 10, nc.vector)
                sp = psS.tile([P, CT, NCH], F32, tag="s", name="sp")
                for half in range(2):
                    nc.tensor.matmul(sp[:, half, :],
                                     lhsT=k_sb[:, :, ts(2 * mp + half, P)],
                                     rhs=q_sb[:, :, nsl],
                                     start=True, stop=True, perf_mode=DR)
                e = work.tile([P, 2, NCH], FP8, tag="e", name="e", bufs=8)
                nc.scalar.activation(out=e, in_=sp, func=AF.Exp,
                                     bias=esh_sb, scale=SCALE)
                pend.append(("av", nch, mp, e))
                while len(pend) > LAG:
                    _pop(pend.pop(0))
                if mp == MP - 1:
                    pend.append(("tailA", nch))
                    pend.append(("tailB", nch))
        for unit in pend:
            _pop(unit)


def build_program():
    nc = bacc.Bacc("TRN2", target_bir_lowering=False, debug=False, num_devices=B)
    d = {}

    def din(name, shape, dt_=F32):
        d[name] = nc.dram_tensor(name, list(shape), dt_, kind="ExternalInput").ap()

    din("x", (C, N))
    din("x_bf", (C, N), mybir.dt.bfloat16)
    din("wqb", (P, CT, C), mybir.dt.bfloat16)
    din("wkb", (P, CT, C), mybir.dt.bfloat16)
    din("wvb", (P, CT, C), mybir.dt.bfloat16)
    din("wo8", (P, CT, C), FP8)
    din("consts_a", (P, 26))
    din("bmask", (G, CT * P))
    out_d = nc.dram_tensor("out", [C, N], F32, kind="ExternalOutput").ap()

    with tile.TileContext(nc) as tc:
        _emit(tc, d, out_d)
    nc.compile()
    return nc


_PROG = None


def _get_program():
    global _PROG
    if _PROG is None:
        _PROG = build_program()
    return _PROG


def make_in_maps(inputs):
    x = np.ascontiguousarray(np.asarray(inputs["x"], dtype=np.float32))
    w_qkv = np.asarray(inputs["w_qkv"], dtype=np.float32)
    b_qkv = np.asarray(inputs["b_qkv"], dtype=np.float32)
    w_out = np.asarray(inputs["w_out"], dtype=np.float32)
    b_out = np.asarray(inputs["b_out"], dtype=np.float32)
    gn_scale = np.asarray(inputs["gn_scale"], dtype=np.float32)
    gn_bias = np.asarray(inputs["gn_bias"], dtype=np.float32)

    fmask = np.zeros((CT, P, G), dtype=np.float32)
    for t in range(CT):
        for p in range(P):
            fmask[t, p, (t * P + p) // GSZ] = 1.0
    # bmask[g, t*P+p] = fmask[t, p, g]
    bmask = np.ascontiguousarray(fmask.transpose(2, 0, 1).reshape(G, CT * P))

    consts_a = np.zeros((P, 26), dtype=np.float32)
    bo_eff = b_out + w_out @ b_qkv[2 * C:3 * C]   # b_v folded (softmax sums to 1)
    for t in range(CT):
        rows = slice(t * P, (t + 1) * P)
        consts_a[:, 0 + t] = b_qkv[0:C][rows]
        consts_a[:, 2 + t] = b_qkv[C:2 * C][rows]
        consts_a[:, 4 + t] = bo_eff[rows]
        consts_a[:, 6 + t] = gn_scale[rows]
        consts_a[:, 8 + t] = gn_bias[rows]
        consts_a[:, 10 + G * t:10 + G * (t + 1)] = fmask[t]

    import ml_dtypes
    E4 = ml_dtypes.float8_e4m3
    BF = ml_dtypes.bfloat16

    def pack(w, dt):
        # [cout, cin] -> lhsT/rhs pair layout [cin_half, 2, cout]
        return np.ascontiguousarray(
            w.T.reshape(CT, P, C).transpose(1, 0, 2)).astype(dt)

    common = {
        "wqb": pack(w_qkv[0:C], BF),
        "wkb": pack(w_qkv[C:2 * C], BF),
        "wvb": pack(w_qkv[2 * C:3 * C], BF),
        "wo8": pack(w_out, E4),
        "consts_a": consts_a,
        "bmask": bmask,
    }
    return [dict(common, x=np.ascontiguousarray(x[b].reshape(C, N)),
                 x_bf=np.ascontiguousarray(x[b].reshape(C, N).astype(BF)))
            for b in range(B)]


def run(inputs, trace=False):
    nc = _get_program()
    in_maps = make_in_maps(inputs)
    res = bass_utils.run_bass_kernel_spmd(nc, in_maps, core_ids=list(range(B)),
                                          trace=trace)
    out = np.stack([res.results[b]["out"] for b in range(B)])
    return out.reshape(B, C, HH, WW), res


def kernel(**inputs):
    out, _ = run(inputs, trace=False)
    return out
